# revision 1
# baseline (speedup 1.0000x reference)
"""Deformable convolution (DCNv1, 3x3, pad=1) on 8 Trainium2 NeuronCores.

Sharding: data-parallel over batch — one sample per core, weights replicated.

Per-core algorithm:
  1. Gather-index math runs TWICE on the vector engine, from two host-side
     layouts of the same fp32 offsets: pass B computes int16 gather indices
     directly in the SWDGE descriptor-ring layout (wrapped in 16 partitions,
     one 32-partition group per queue), so no on-device shuffle of indices is
     needed; pass A computes bilinear corner weights in pixel-major layout.
     Both passes see bit-identical inputs, so their floor() results agree.
  2. One dma_gather descriptor per (tap, pixel) fetches the full 2x2 bilinear
     patch (512 fp16 values) from a row-pair-interleaved channels-last copy
     of the image in DRAM. Calls rotate over the 4 SWDGE queues; each queue
     has a dedicated Q7 core pair, so 4 calls generate descriptors in
     parallel. Index/weight pass slices are emitted just-in-time between the
     first units so no engine queue head-blocks another at startup.
  3. Bilinear blend + transpose on the PE: per pixel block, 4 corner matmuls
     against weighted-diagonal matrices accumulate the blended, transposed
     im2col columns in fp32 PSUM. The diagonals are built in one DVE
     multiply per unit that hits the 2x_1P fast mode (pair-duplicated
     weights against a dense tiled identity, both step-1).
  4. Conv = 9 accumulated fp16 matmuls into fp32 PSUM; bias on evacuation.

Numerics: gather/blend/cols/weights in fp16, PSUM accumulation fp32.
Empirical end-to-end rel err vs fp32 reference: ~4e-4.
"""
from contextlib import ExitStack

import numpy as np

import concourse.bass as bass
import concourse.bacc as bacc
import concourse.tile as tile
from concourse import mybir
from concourse.bass import AP
from concourse import library_config
from concourse.bass_utils import run_bass_kernel_spmd

F32 = mybir.dt.float32
F16 = mybir.dt.float16
I32 = mybir.dt.int32
I16 = mybir.dt.int16

KH = KW = 3
K = 9
H = W = 64
HW = H * W
C = 128
O = 128
PAD_PX = 65
NV = 4352
TOT_PX = 4480
GELEM = 512          # one 2x2 patch: [x00|x10|x01|x11], fp16
GSTEP = 256          # slot stride (one pixel-row-pair slot)
NB = 32
CHUNKS = 4
NBC = NB // CHUNKS   # 8 blocks/chunk
PXC = HW // CHUNKS   # 1024 px/chunk
NCALL = CHUNKS * K   # 36 gather calls, 1024 idx each
NQ = 4               # SWDGE queues
CPQ = NCALL // NQ    # 9 calls per queue
BSLICE = 3           # pass-B column slices (pipeline index math vs gathers)

# corner order matches the gathered patch layout: slot ci = dx*2 + dy
CORNERS = ((0, 0), (1, 0), (0, 1), (1, 1))  # (dy, dx) for ci = 0..3


def _make_base_flat() -> np.ndarray:
    """[18, HW] fp32: (dy_base, dx_base) rows per tap, +1 pre-shift baked."""
    p = np.arange(HW)
    py = (p // W).astype(np.float32)
    px = (p % W).astype(np.float32)
    base = np.empty((18, HW), np.float32)
    for ki in range(KH):
        for kj in range(KW):
            k = ki * KW + kj
            base[2 * k] = py + ki
            base[2 * k + 1] = px + kj
    return base


def _wrap_ring_layout(flat18: np.ndarray) -> np.ndarray:
    """[18, HW] -> [128, CPQ*64, 2] in SWDGE ring layout.

    Partition p = 16*g + r (g = core group, r = ring partition). Groups
    2q, 2q+1 hold queue q's calls. Within a queue, call cq covers columns
    [cq*64, cq*64+64), ordered (b_local, t) so that gather idx number
    n = b_local*128 + 16*t + r sits at (partition r, column n//16).
    Call c (global order c = ch*K + k) runs on queue c % 4 as call c // 4.
    """
    out = np.zeros((128, CPQ * 64, 2), np.float32)
    # [tap-pair, b, t, r]
    a = flat18.reshape(18, NB, 8, 16)
    for c in range(NCALL):
        ch, k = divmod(c, K)
        q, cq = c % NQ, c // NQ
        # [2, NBC, 8, 16] -> [16(r), NBC*8(b_local,t), 2]
        blk = a[2 * k:2 * k + 2, ch * NBC:(ch + 1) * NBC]
        blk = blk.transpose(3, 1, 2, 0).reshape(16, NBC * 8, 2)
        for g in (2 * q, 2 * q + 1):
            out[16 * g:16 * g + 16, cq * 64:cq * 64 + 64] = blk
    return np.ascontiguousarray(out)


def _prep_core_inputs(x_b, offset_b, weight, bias, consts) -> dict:
    xclb = np.zeros((TOT_PX + W, C), np.float16)
    xclb[PAD_PX:PAD_PX + HW] = x_b.reshape(C, HW).T.astype(np.float16)
    xcl = np.zeros((TOT_PX, 2 * C), np.float16)
    xcl[:, :C] = xclb[:TOT_PX]
    xcl[:, C:] = xclb[W:TOT_PX + W]
    off_flat = np.ascontiguousarray(offset_b.reshape(18, HW)).astype(np.float32)
    offs = np.ascontiguousarray(
        off_flat.reshape(18, NB, 128).transpose(2, 0, 1))
    wts = np.ascontiguousarray(
        weight.reshape(O, C, K).transpose(2, 1, 0)).astype(np.float16)
    return {
        "xcl": xcl,
        "offs": offs,
        "base": consts["base"],
        "offs2": _wrap_ring_layout(off_flat),
        "base2": consts["base2"],
        "wts": wts,
        "bias_in": bias.reshape(O, 1).astype(np.float32),
        "identw_in": np.tile(np.eye(128, dtype=np.float16), (1, 4 * NBC)),
    }


def _dcn_core_kernel(tc, outs, ins):
    nc = tc.nc
    out_d = outs["out"]

    with ExitStack() as ctx:
        consts = ctx.enter_context(tc.tile_pool(name="consts", bufs=1))
        idxbp = ctx.enter_context(tc.tile_pool(name="idxb", bufs=1))
        idxp = ctx.enter_context(tc.tile_pool(name="idx", bufs=1))
        gath = ctx.enter_context(tc.tile_pool(name="gath", bufs=6))
        pmp = ctx.enter_context(tc.tile_pool(name="pm", bufs=5))
        colp = ctx.enter_context(tc.tile_pool(name="col", bufs=3))
        outp = ctx.enter_context(tc.tile_pool(name="outsb", bufs=2))
        psums = ctx.enter_context(tc.tile_pool(name="psums", bufs=4, space="PSUM"))
        psumc = ctx.enter_context(tc.tile_pool(name="psumc", bufs=2, space="PSUM"))

        # ring-layout offsets first: pass B is the critical path to gathers;
        # per-slice tiles so slice 0's math starts as soon as its data lands
        NC2 = CPQ * 64
        CQSL = ((0, 1), (1, 5), (5, 7), (7, 9))  # cq ranges per pass-B slice
        offs2_t = [consts.tile([128, (b - a) * 64, 2], F32, name=f"offs2_{a}")
                   for a, b in CQSL]
        base2_t = [consts.tile([128, (b - a) * 64, 2], F32, name=f"base2_{a}")
                   for a, b in CQSL]
        for s, (a, b) in enumerate(CQSL):
            nc.sync.dma_start(out=offs2_t[s],
                              in_=ins["offs2"][:, a * 64:b * 64, :])
            nc.sync.dma_start(out=base2_t[s],
                              in_=ins["base2"][:, a * 64:b * 64, :])
        offs = consts.tile([128, K, 2, NB], F32)
        base = consts.tile([128, K, 2, NB], F32)
        nc.scalar.dma_start(out=offs, in_=ins["offs"])
        nc.scalar.dma_start(out=base, in_=ins["base"])
        wts = consts.tile([128, K, O], F16)
        for k in range(K):
            nc.scalar.dma_start(out=wts[:, k, :], in_=ins["wts"][k])
        bias_sb = consts.tile([128, 1], F32)
        nc.scalar.dma_start(out=bias_sb, in_=ins["bias_in"])
        identw = consts.tile([128, 4 * NBC * 128], F16)
        nc.scalar.dma_start(out=identw, in_=ins["identw_in"])
        nc.gpsimd.load_library(library_config.mlp)

        # ---- pass B: gather indices, computed directly in SWDGE ring layout.
        # Sliced by column group, each slice writing its own idxb tile so
        # early gather calls unblock as soon as their slice lands (tile deps
        # are whole-tile). Value semantics must match pass A exactly (same
        # fp32 inputs, same clamp, same trunc) so both agree on floor().
        pos2 = idxbp.tile([128, NC2, 2], F32)
        fi2 = idxbp.tile([128, NC2, 2], I32)
        fif2 = idxbp.tile([128, NC2, 2], F32)
        gt2 = idxbp.tile([128, NC2, 2], F32)
        gidx2 = idxbp.tile([128, NC2], F32)
        idxb_t = [idxbp.tile([128, (b - a) * 64], I16, name=f"idxb{a}")
                  for a, b in CQSL]

        def pass_b_slice(s):
            a, b = CQSL[s]
            sl = slice(a * 64, b * 64)
            nc.vector.tensor_tensor(out=pos2[:, sl, :], in0=offs2_t[s][:, :, :],
                                    in1=base2_t[s][:, :, :],
                                    op=mybir.AluOpType.add)
            nc.vector.tensor_scalar(out=pos2[:, sl, :], in0=pos2[:, sl, :],
                                    scalar1=0.0, scalar2=65.0,
                                    op0=mybir.AluOpType.max,
                                    op1=mybir.AluOpType.min)
            nc.vector.tensor_copy(out=fi2[:, sl, :], in_=pos2[:, sl, :])
            nc.vector.tensor_copy(out=fif2[:, sl, :], in_=fi2[:, sl, :])
            # int conversion rounds to nearest; correct to floor (must match
            # pass A bit-for-bit)
            nc.vector.tensor_tensor(out=gt2[:, sl, :], in0=fif2[:, sl, :],
                                    in1=pos2[:, sl, :], op=mybir.AluOpType.is_gt)
            nc.vector.tensor_tensor(out=fif2[:, sl, :], in0=fif2[:, sl, :],
                                    in1=gt2[:, sl, :],
                                    op=mybir.AluOpType.subtract)
            nc.vector.tensor_scalar(out=gidx2[:, sl], in0=fif2[:, sl, 0],
                                    scalar1=64.0, scalar2=None,
                                    op0=mybir.AluOpType.mult)
            nc.vector.tensor_tensor(out=gidx2[:, sl], in0=gidx2[:, sl],
                                    in1=fif2[:, sl, 1], op=mybir.AluOpType.add)
            nc.vector.tensor_copy(out=idxb_t[s], in_=gidx2[:, sl])

        def idxb_slice_ap(cq):
            for s, (a, b) in enumerate(CQSL):
                if a <= cq < b:
                    return idxb_t[s][:, (cq - a) * 64:(cq - a) * 64 + 64]
            raise AssertionError(cq)

        # ---- pass A: bilinear corner weights, pixel-major (fp32), sliced by
        # tap triple so the first dk ops don't wait for the full pass
        pos = idxp.tile([128, K, 2, NB], F32)
        fi = idxp.tile([128, K, 2, NB], I32)
        fint = idxp.tile([128, K, 2, NB], F32)
        gt = idxp.tile([128, K, 2, NB], F32)
        frac = idxp.tile([128, K, 2, NB], F32)
        v0 = idxp.tile([128, K, 2, NB], F32)
        v1 = idxp.tile([128, K, 2, NB], F32)
        w0 = idxp.tile([128, K, 2, NB], F32)
        w1 = idxp.tile([128, K, 2, NB], F32)
        ASL = 3
        kts = K // ASL
        w4_t = [idxp.tile([128, kts, NB, 4], F16, name=f"w4_{s}")
                for s in range(ASL)]
        # w4 values duplicated into adjacent pairs: lets the dk construction
        # read the weight operand with a step-1 inner dim (DVE 2x_1P fast
        # mode) instead of a step-0 broadcast (1x mode)
        w4x2_t = [idxp.tile([128, kts, NB, 4, 2], F16, name=f"w4x2_{s}")
                  for s in range(ASL)]

        def pass_a_slice(s):
            ks = slice(s * kts, (s + 1) * kts)
            nc.vector.tensor_tensor(out=pos[:, ks], in0=offs[:, ks],
                                    in1=base[:, ks], op=mybir.AluOpType.add)
            nc.vector.tensor_scalar(out=pos[:, ks], in0=pos[:, ks], scalar1=0.0,
                                    scalar2=65.0, op0=mybir.AluOpType.max,
                                    op1=mybir.AluOpType.min)
            nc.vector.tensor_copy(out=fi[:, ks], in_=pos[:, ks])
            nc.vector.tensor_copy(out=fint[:, ks], in_=fi[:, ks])
            nc.vector.tensor_tensor(out=gt[:, ks], in0=fint[:, ks],
                                    in1=pos[:, ks], op=mybir.AluOpType.is_gt)
            nc.vector.tensor_tensor(out=fint[:, ks], in0=fint[:, ks],
                                    in1=gt[:, ks], op=mybir.AluOpType.subtract)
            nc.vector.tensor_tensor(out=frac[:, ks], in0=pos[:, ks],
                                    in1=fint[:, ks], op=mybir.AluOpType.subtract)
            nc.vector.tensor_scalar(out=v0[:, ks], in0=fint[:, ks], scalar1=1.0,
                                    scalar2=None, op0=mybir.AluOpType.is_ge)
            nc.vector.tensor_scalar(out=v1[:, ks], in0=fint[:, ks], scalar1=64.0,
                                    scalar2=None, op0=mybir.AluOpType.is_le)
            nc.vector.tensor_tensor(out=v0[:, ks], in0=v0[:, ks], in1=v1[:, ks],
                                    op=mybir.AluOpType.mult)
            nc.vector.tensor_scalar(out=v1[:, ks], in0=fint[:, ks], scalar1=63.0,
                                    scalar2=None, op0=mybir.AluOpType.is_le)
            nc.vector.tensor_scalar(out=w0[:, ks], in0=frac[:, ks], scalar1=-1.0,
                                    scalar2=1.0, op0=mybir.AluOpType.mult,
                                    op1=mybir.AluOpType.add)
            nc.vector.tensor_tensor(out=w0[:, ks], in0=w0[:, ks], in1=v0[:, ks],
                                    op=mybir.AluOpType.mult)
            nc.vector.tensor_tensor(out=w1[:, ks], in0=frac[:, ks],
                                    in1=v1[:, ks], op=mybir.AluOpType.mult)
            wy = (w0, w1)
            wx = (w0, w1)
            for ci, (dy, dx) in enumerate(CORNERS):
                nc.vector.tensor_tensor(
                    out=w4_t[s][:, :, :, ci], in0=wy[dy][:, ks, 0, :],
                    in1=wx[dx][:, ks, 1, :], op=mybir.AluOpType.mult)
            # pair-duplication copy runs on the scalar engine: the vector
            # engine is the binding resource during the startup hump
            wsrc = w4_t[s][:, :, :, :]
            w4src = bass.AP(tensor=wsrc.tensor, offset=wsrc.offset,
                            ap=[wsrc.ap[0], [1, kts * 4 * NB], [0, 2]])
            wdst = w4x2_t[s][:, :, :, :, :]
            w4dst = bass.AP(tensor=wdst.tensor, offset=wdst.offset,
                            ap=[wdst.ap[0], [2, kts * 4 * NB], [1, 2]])
            nc.scalar.copy(out=w4dst, in_=w4src)

        # interleave: micro B slice unblocks the init-paying first gather,
        # B1 covers calls 4..19, A0 unblocks the first dk ops. A1/A2/B2 are
        # emitted just-in-time between the first units (the vector engine
        # executes in program order — emitting all passes up front would
        # park every dk op behind them and starve the PE until ~40us).
        pass_b_slice(0)
        pass_b_slice(1)
        pass_a_slice(0)

        xview = AP(tensor=ins["xcl"].tensor, offset=0,
                   ap=[[GSTEP, NV], [1, GELEM]])

        for ch in range(CHUNKS):
            conv_ps = psumc.tile([128, PXC], F32, space="PSUM")
            bs = ch * NBC
            for k in range(K):
                c = ch * K + k
                if c == 3:
                    pass_a_slice(1)
                elif c == 4:
                    pass_b_slice(2)
                elif c == 5:
                    pass_b_slice(3)
                elif c == 6:
                    pass_a_slice(2)
                gk = gath.tile([128, NBC, GELEM], F16)
                # one call per (chunk, tap); queue = c % 4 has a dedicated
                # Q7 core pair, so 4 calls' descriptor generation overlaps
                nc.gpsimd.dma_gather(
                    out_ap=gk[:, :, :],
                    in_ap=xview,
                    idxs_ap=idxb_slice_ap(c // NQ),
                    num_idxs=NBC * 128,
                    num_idxs_reg=NBC * 128,
                    elem_size=GELEM,
                    elem_step=GSTEP,
                    queue_num=c % NQ,
                )
                # weighted-diagonal moving operands: Dk[q, ci, b, j] =
                # ident[q, j] * w4[q, k, ci, bs+b]. Both operands read with
                # step-1 inner dims (identw is a dense tiled identity, w4x2
                # holds each weight as an adjacent pair) so the DVE multiply
                # runs in 2x_1P mode. The corner SUM then rides the PE's fp32
                # PSUM accumulation, so fp16 rounding only touches the inputs.
                dk = pmp.tile([128, NBC, 4, C], F16)
                dv = dk[:, :, :, :]
                dk_f = bass.AP(tensor=dv.tensor, offset=dv.offset,
                               ap=[dv.ap[0], [1, 4 * NBC * C]])
                wv = w4x2_t[k // kts][:, k % kts, bs:bs + NBC, :, :]
                w_p = bass.AP(tensor=wv.tensor, offset=wv.offset,
                              ap=[wv.ap[0], [2, 4 * NBC], [0, C // 2], [1, 2]])
                nc.vector.tensor_tensor(out=dk_f, in0=identw[:, :], in1=w_p,
                                        op=mybir.AluOpType.mult)
                # per pixel block: psum[c, j] += sum_ci gk_ci.T @ diag(w_ci)
                colk = colp.tile([128, PXC], F16)
                for bg in range(NBC // 4):
                    pst = psums.tile([128, 512], F32, space="PSUM")
                    for j in range(4):
                        b = bg * 4 + j
                        for ci in range(4):
                            nc.tensor.matmul(
                                out=pst[:, j * 128:(j + 1) * 128],
                                lhsT=gk[:, b, ci * C:(ci + 1) * C],
                                rhs=dk[:, b, ci, :],
                                start=(ci == 0), stop=(ci == 3))
                    nc.scalar.copy(out=colk[:, bg * 512:(bg + 1) * 512], in_=pst)
                for m in range(PXC // 512):
                    nc.tensor.matmul(
                        out=conv_ps[:, m * 512:(m + 1) * 512],
                        lhsT=wts[:, k, :],
                        rhs=colk[:, m * 512:(m + 1) * 512],
                        start=(k == 0), stop=(k == K - 1))
            out_sb = outp.tile([128, PXC], F32)
            nc.scalar.activation(out=out_sb, in_=conv_ps,
                                 func=mybir.ActivationFunctionType.Identity,
                                 bias=bias_sb[:, :], scale=1.0)
            nc.sync.dma_start(out=out_d[:, ch * PXC:(ch + 1) * PXC], in_=out_sb)


_IN_SPECS = {
    "xcl": ((TOT_PX, 2 * C), np.float16),
    "offs": ((128, 18, NB), np.float32),
    "base": ((128, 18, NB), np.float32),
    "offs2": ((128, CPQ * 64, 2), np.float32),
    "base2": ((128, CPQ * 64, 2), np.float32),
    "wts": ((K, C, O), np.float16),
    "bias_in": ((O, 1), np.float32),
    "identw_in": ((128, 4 * NBC * 128), np.float16),
}

_prog_cache = {}


def _build_program():
    if "nc" in _prog_cache:
        return _prog_cache["nc"]
    nc = bacc.Bacc("TRN2", target_bir_lowering=False, debug=False,
                   num_swdge_queues=4)
    ins = {}
    for name, (shape, dtype) in _IN_SPECS.items():
        ins[name] = nc.dram_tensor(
            name, list(shape), mybir.dt.from_np(np.dtype(dtype)),
            kind="ExternalInput").ap()
    outs = {"out": nc.dram_tensor("out", [O, HW], F32,
                                  kind="ExternalOutput").ap()}
    with tile.TileContext(nc) as tc:
        _dcn_core_kernel(tc, outs, ins)
    nc.compile()
    _prog_cache["nc"] = nc
    return nc


def run_dcn(x, offset, weight, bias, trace=False):
    x = np.ascontiguousarray(x, dtype=np.float32)
    offset = np.ascontiguousarray(offset, dtype=np.float32)
    weight = np.ascontiguousarray(weight, dtype=np.float32)
    bias = np.ascontiguousarray(bias, dtype=np.float32)
    B = x.shape[0]
    base_flat = _make_base_flat()
    consts = {
        "base": np.ascontiguousarray(
            base_flat.reshape(18, NB, 128).transpose(2, 0, 1)),
        "base2": _wrap_ring_layout(base_flat),
    }
    in_maps = [_prep_core_inputs(x[b], offset[b], weight, bias, consts)
               for b in range(B)]
    nc = _build_program()
    res = run_bass_kernel_spmd(nc, in_maps, core_ids=list(range(B)), trace=trace)
    out = np.stack([r["out"] for r in res.results]).reshape(B, O, H, W)
    return out, res


def kernel(x, offset, weight, bias):
    out, _ = run_dcn(x, offset, weight, bias)
    return out.astype(np.float32)

